# revision 8
# baseline (speedup 1.0000x reference)
"""DRQN fused kernel for 8 TRN2 NeuronCores.

Data-parallel over batch B=1024 -> 128 rows per core, L=6 timesteps.
Per core the whole net runs locally (no collectives):

    inp = concat(x, pos_onehot)      -- pos is constant (l0=0), so it is
                                        folded into the first-layer biases
    att = sigmoid(relu(inp@aw1+b)@aw2+b)       [768, 1]
    enc = relu(inp@ew1+b)@ew2+b                [768, 1024]
    out = cumsum_L(enc*att)                    [768, 1024]
    act = relu(relu(out@qw1+b)@qw2+b)@qw3+b    [768, 12972]

All weights and activations are fp16 (PE runs fp16 at the same rate as
f32r, so this halves HBM/SBUF traffic at no compute cost; fp16's 10-bit
mantissa keeps the error ~1e-3 against the 2e-2 gate).  PSUM accumulation
is fp32; the gate + cumsum chain is kept in fp32 and rounded to fp16 once.
x is transposed to [feature, row] on the host so no PE transposes are
needed.  qw3 is re-tiled on the host into [26, 128, 8*512] fp16 blobs so
each column tile is one contiguous 8KB/partition DMA.

Per-core roofline: 26.2 GFLOP -> ~334 us on the PE at 2.4 GHz full rate;
~74 MB of HBM traffic -> ~207 us at 358 GB/s.  PE-bound.
"""

import numpy as np

import concourse.bass as bass
import concourse.mybir as mybir
from concourse import bacc
from concourse.bass_utils import run_bass_kernel_spmd
from concourse.tile import TileContext

F32 = mybir.dt.float32
F32R = mybir.dt.float32r
F16 = mybir.dt.float16

L, B, N = 6, 1024, 512
G = 6
H, AH, ACT = 1024, 256, 12972
NCORES = 8
BL = B // NCORES          # 128 rows of batch per core
R = L * BL                # 768 rows per core
RG = 2                    # row groups
RGS = R // RG             # 384 rows per group

KN = N // 128             # 4 K-chunks of x features
KH = H // 128             # 8 chunks of hidden features
KA = AH // 128            # 2 chunks of attention features
MROW = R // 128           # 6 row chunks (row chunk m == timestep l)

NT = 26                   # qw3 column tiles of 512 (last holds 172 valid)
ACTP = NT * 512           # 13312 padded action dim
LAST_NN = ACT - (NT - 1) * 512   # 172

BUFS = dict(mm=4, att=2, qw3s=3, qb3r=2, qb3bc=3, pq=8, st=8)
REPS = 1


def build_nc():
    nc = bacc.Bacc()

    xt_h = nc.declare_dram_parameter("xT", [N, R], F16, isOutput=False)
    aw1_h = nc.declare_dram_parameter("aw1r", [128, KN * AH], F16, isOutput=False)
    ab1_h = nc.declare_dram_parameter("ab1e", [AH], F32, isOutput=False)
    aw2_h = nc.declare_dram_parameter("aw2r", [128, KA], F16, isOutput=False)
    ab2_h = nc.declare_dram_parameter("ab2", [1], F32, isOutput=False)
    ew1_h = nc.declare_dram_parameter("ew1", [N, H], F16, isOutput=False)
    eb1_h = nc.declare_dram_parameter("eb1e", [H], F32, isOutput=False)
    ew2_h = nc.declare_dram_parameter("ew2", [H, H], F16, isOutput=False)
    eb2_h = nc.declare_dram_parameter("eb2", [H], F32, isOutput=False)
    qw1_h = nc.declare_dram_parameter("qw1", [H, H], F16, isOutput=False)
    qb1_h = nc.declare_dram_parameter("qb1", [H], F32, isOutput=False)
    qw2_h = nc.declare_dram_parameter("qw2", [H, H], F16, isOutput=False)
    qb2_h = nc.declare_dram_parameter("qb2", [H], F32, isOutput=False)
    qw3_h = nc.declare_dram_parameter("qw3r", [NT, 128, KH * 512], F16,
                                      isOutput=False)
    qb3_h = nc.declare_dram_parameter("qb3p", [ACTP], F32R, isOutput=False)
    ones_h = nc.declare_dram_parameter("ones", [1, 128], F32R, isOutput=False)
    out_h = nc.declare_dram_parameter("out", [L, BL, ACT], F32, isOutput=True)

    from contextlib import nullcontext
    with TileContext(nc) as tc:
      # REPS>1 wraps the body in a hardware loop — used only for timing
      # (amortizes the ~11ms per-dispatch tunnel overhead); the graded
      # path builds with REPS=1 and no loop.
      with (tc.For_i(0, REPS) if REPS > 1 else nullcontext()):
        persist = tc.alloc_tile_pool(name="persist", bufs=1)
        ps_a = tc.alloc_tile_pool(name="ps_a", bufs=1, space="PSUM")
        # released in reverse alloc order (LIFO per memory space)
        pool_h = tc.alloc_tile_pool(name="pool_h", bufs=1)   # h1t, qw2
        pool_g = tc.alloc_tile_pool(name="pool_g", bufs=1)   # gt, gtb, qw1
        pool_e = tc.alloc_tile_pool(name="pool_e", bufs=1)   # e1t, ew2
        pool_x = tc.alloc_tile_pool(name="pool_x", bufs=1)   # xt, aw1, ew1, a1t

        # ---- constants / biases ----
        ones_t = persist.tile([1, 128], F32R, name="ones_t")
        nc.sync.dma_start(out=ones_t, in_=ones_h[:])
        ab1_t = persist.tile([128, KA], F32, name="ab1_t")
        nc.sync.dma_start(out=ab1_t, in_=ab1_h[:].rearrange("(m p) -> p m", p=128))
        eb1_t = persist.tile([128, KH], F32, name="eb1_t")
        nc.sync.dma_start(out=eb1_t, in_=eb1_h[:].rearrange("(m p) -> p m", p=128))
        eb2_t = persist.tile([128, KH], F32, name="eb2_t")
        nc.sync.dma_start(out=eb2_t, in_=eb2_h[:].rearrange("(m p) -> p m", p=128))
        qb1_t = persist.tile([128, KH], F32, name="qb1_t")
        nc.sync.dma_start(out=qb1_t, in_=qb1_h[:].rearrange("(m p) -> p m", p=128))
        qb2_t = persist.tile([128, KH], F32, name="qb2_t")
        nc.sync.dma_start(out=qb2_t, in_=qb2_h[:].rearrange("(m p) -> p m", p=128))
        ab2_t = persist.tile([1, 1], F32, name="ab2_t")
        nc.sync.dma_start(out=ab2_t, in_=ab2_h[:])

        att_s = persist.tile([1, R], F32R, name="att_s")
        att_bc = persist.tile([128, R], F32, name="att_bc")
        h2t = [persist.tile([128, R], F16, name=f"h2t{k}", tag="h2t", bufs=KH)
               for k in range(KH)]

        # ---- input / weight loads (x is pre-transposed on the host) ----
        xt = [pool_x.tile([128, R], F16, name=f"xt{k}", tag="xt", bufs=KN)
              for k in range(KN)]
        for k in range(KN):
            nc.sync.dma_start(out=xt[k], in_=xt_h[k * 128:(k + 1) * 128, :])

        aw1_t = pool_x.tile([128, KN * AH], F16, name="aw1_t")
        nc.sync.dma_start(out=aw1_t, in_=aw1_h[:])
        aw2_t = pool_x.tile([128, KA], F16, name="aw2_t")
        nc.sync.dma_start(out=aw2_t, in_=aw2_h[:])
        ew1_t = [pool_x.tile([128, H], F16, name=f"ew1t{k}", tag="ew1t", bufs=KN)
                 for k in range(KN)]
        for k in range(KN):
            nc.sync.dma_start(out=ew1_t[k], in_=ew1_h[k * 128:(k + 1) * 128, :])
        ew2_t = [pool_e.tile([128, H], F16, name=f"ew2t{k}", tag="ew2t", bufs=KH)
                 for k in range(KH)]
        for k in range(KH):
            nc.sync.dma_start(out=ew2_t[k], in_=ew2_h[k * 128:(k + 1) * 128, :])
        qw1_t = [pool_g.tile([128, H], F16, name=f"qw1t{k}", tag="qw1t", bufs=KH)
                 for k in range(KH)]
        for k in range(KH):
            nc.sync.dma_start(out=qw1_t[k], in_=qw1_h[k * 128:(k + 1) * 128, :])
        qw2_t = [pool_h.tile([128, H], F16, name=f"qw2t{k}", tag="qw2t", bufs=KH)
                 for k in range(KH)]
        for k in range(KH):
            nc.sync.dma_start(out=qw2_t[k], in_=qw2_h[k * 128:(k + 1) * 128, :])

        # ---- attention branch layer 1 ----
        a1t = [pool_x.tile([128, R], F16, name=f"a1t{m}", tag="a1t", bufs=KA)
               for m in range(KA)]
        for m in range(KA):
            pls = [ps_a.tile([128, RGS], F32, name="a1ps", tag="mm", bufs=BUFS["mm"])
                   for _ in range(RG)]
            for k in range(KN):
                for g in range(RG):
                    nc.tensor.matmul(
                        pls[g], aw1_t[:, k * AH + m * 128:k * AH + (m + 1) * 128],
                        xt[k][:, g * RGS:(g + 1) * RGS],
                        start=(k == 0), stop=(k == KN - 1))
            for g in range(RG):
                nc.scalar.activation(
                    a1t[m][:, g * RGS:(g + 1) * RGS], pls[g],
                    mybir.ActivationFunctionType.Relu, bias=ab1_t[:, m:m + 1])

        # ---- attention layer 2 + sigmoid + broadcast ----
        for g in range(RG):
            aps = ps_a.tile([1, RGS], F32, name="aps", tag="att", bufs=BUFS["att"])
            for k in range(KA):
                nc.tensor.matmul(
                    aps, aw2_t[:, k:k + 1], a1t[k][:, g * RGS:(g + 1) * RGS],
                    start=(k == 0), stop=(k == KA - 1))
            nc.scalar.activation(
                att_s[:, g * RGS:(g + 1) * RGS], aps,
                mybir.ActivationFunctionType.Sigmoid, bias=ab2_t[:, 0:1])
            bps = ps_a.tile([128, RGS], F32, name="bps", tag="att", bufs=BUFS["att"])
            nc.tensor.matmul(
                bps, ones_t, att_s[:, g * RGS:(g + 1) * RGS],
                start=True, stop=True)
            nc.vector.tensor_copy(att_bc[:, g * RGS:(g + 1) * RGS], bps)

        # ---- encoder layer 1 ----
        e1t = [pool_e.tile([128, R], F16, name=f"e1t{m}", tag="e1t", bufs=KH)
               for m in range(KH)]
        for m in range(KH):
            pls = [ps_a.tile([128, RGS], F32, name="e1ps", tag="mm", bufs=BUFS["mm"])
                   for _ in range(RG)]
            for k in range(KN):
                for g in range(RG):
                    nc.tensor.matmul(
                        pls[g], ew1_t[k][:, m * 128:(m + 1) * 128],
                        xt[k][:, g * RGS:(g + 1) * RGS],
                        start=(k == 0), stop=(k == KN - 1))
            for g in range(RG):
                nc.scalar.activation(
                    e1t[m][:, g * RGS:(g + 1) * RGS], pls[g],
                    mybir.ActivationFunctionType.Relu, bias=eb1_t[:, m:m + 1])

        pool_x.release()

        # ---- encoder layer 2 + gate + cumsum over L (fp32 master) ----
        gt = [pool_g.tile([128, R], F32, name=f"gt{m}", tag="gt", bufs=KH)
              for m in range(KH)]
        gtb = [pool_g.tile([128, R], F16, name=f"gtb{m}", tag="gtb", bufs=KH)
               for m in range(KH)]
        for m in range(KH):
            pls = [ps_a.tile([128, RGS], F32, name="e2ps", tag="mm", bufs=BUFS["mm"])
                   for _ in range(RG)]
            for k in range(KH):
                for g in range(RG):
                    nc.tensor.matmul(
                        pls[g], ew2_t[k][:, m * 128:(m + 1) * 128],
                        e1t[k][:, g * RGS:(g + 1) * RGS],
                        start=(k == 0), stop=(k == KH - 1))
            for g in range(RG):
                # gt = (psum + eb2) * att
                nc.vector.scalar_tensor_tensor(
                    gt[m][:, g * RGS:(g + 1) * RGS], pls[g], eb2_t[:, m:m + 1],
                    att_bc[:, g * RGS:(g + 1) * RGS],
                    op0=mybir.AluOpType.add, op1=mybir.AluOpType.mult)
            # running sum over the 6 timesteps (128-row blocks of free dim)
            for l in range(1, L):
                nc.vector.tensor_add(
                    gt[m][:, l * 128:(l + 1) * 128],
                    gt[m][:, l * 128:(l + 1) * 128],
                    gt[m][:, (l - 1) * 128:l * 128])
            nc.scalar.copy(gtb[m], gt[m])

        pool_e.release()

        # ---- q head layer 1 ----
        h1t = [pool_h.tile([128, R], F16, name=f"h1t{m}", tag="h1t", bufs=KH)
               for m in range(KH)]
        for m in range(KH):
            pls = [ps_a.tile([128, RGS], F32, name="q1ps", tag="mm", bufs=BUFS["mm"])
                   for _ in range(RG)]
            for k in range(KH):
                for g in range(RG):
                    nc.tensor.matmul(
                        pls[g], qw1_t[k][:, m * 128:(m + 1) * 128],
                        gtb[k][:, g * RGS:(g + 1) * RGS],
                        start=(k == 0), stop=(k == KH - 1))
            for g in range(RG):
                nc.scalar.activation(
                    h1t[m][:, g * RGS:(g + 1) * RGS], pls[g],
                    mybir.ActivationFunctionType.Relu, bias=qb1_t[:, m:m + 1])

        pool_g.release()

        # ---- q head layer 2 ----
        for m in range(KH):
            pls = [ps_a.tile([128, RGS], F32, name="q2ps", tag="mm", bufs=BUFS["mm"])
                   for _ in range(RG)]
            for k in range(KH):
                for g in range(RG):
                    nc.tensor.matmul(
                        pls[g], qw2_t[k][:, m * 128:(m + 1) * 128],
                        h1t[k][:, g * RGS:(g + 1) * RGS],
                        start=(k == 0), stop=(k == KH - 1))
            for g in range(RG):
                nc.scalar.activation(
                    h2t[m][:, g * RGS:(g + 1) * RGS], pls[g],
                    mybir.ActivationFunctionType.Relu, bias=qb2_t[:, m:m + 1])

        pool_h.release()
        ps_a.release()

        # ---- action layer: out[m*128+b, :] = h2 @ qw3 + qb3 ----
        pool_q3 = tc.alloc_tile_pool(name="pool_q3", bufs=1)
        ps_b = tc.alloc_tile_pool(name="ps_b", bufs=1, space="PSUM")

        for nt in range(NT):
            nn = 512 if nt < NT - 1 else LAST_NN
            n0 = nt * 512
            wts = pool_q3.tile([128, KH * 512], F16, name="qw3s", tag="qw3s",
                               bufs=BUFS["qw3s"])
            nc.sync.dma_start(out=wts, in_=qw3_h[nt])

            qb3r = pool_q3.tile([1, 512], F32R, name="qb3r", tag="qb3r",
                                bufs=BUFS["qb3r"])
            nc.sync.dma_start(out=qb3r, in_=qb3_h[nt * 512:(nt + 1) * 512][None, :])
            bps = ps_b.tile([128, 512], F32, name="q3bps", tag="pq", bufs=BUFS["pq"])
            nc.tensor.matmul(bps, ones_t, qb3r, start=True, stop=True)
            qb3bc = pool_q3.tile([128, 512], F32, name="qb3bc", tag="qb3bc",
                                 bufs=BUFS["qb3bc"])
            nc.vector.tensor_copy(qb3bc, bps)

            for m in range(MROW):
                pq = ps_b.tile([128, 512], F32, name="pq", tag="pq", bufs=BUFS["pq"])
                for k in range(KH):
                    nc.tensor.matmul(
                        pq, h2t[k][:, m * 128:(m + 1) * 128],
                        wts[:, k * 512:(k + 1) * 512],
                        start=(k == 0), stop=(k == KH - 1))
                st = pool_q3.tile([128, 512], F32, name="st", tag="st",
                                  bufs=BUFS["st"])
                nc.vector.tensor_add(st, pq, qb3bc)
                nc.sync.dma_start(out=out_h[m, :, n0:n0 + nn], in_=st[:, :nn])

        pool_q3.release()
        ps_b.release()
        persist.release()

    nc.finalize()
    return nc


_NC_CACHE = {}


def _get_nc(reps=1):
    global REPS
    if reps not in _NC_CACHE:
        old = REPS
        REPS = reps
        try:
            _NC_CACHE[reps] = build_nc()
        finally:
            REPS = old
    return _NC_CACHE[reps]


def _prep_in_maps(inputs):
    f = lambda a: np.ascontiguousarray(np.asarray(a, dtype=np.float32))
    h = lambda a: np.ascontiguousarray(a.astype(np.float16))
    x = f(inputs["x"])
    aw1 = f(inputs["aw1"])
    ab1e = f(inputs["ab1"]) + aw1[N]          # fold pos one-hot (l0=0)
    ew1 = f(inputs["ew1"])
    eb1e = f(inputs["eb1"]) + ew1[N]

    aw1u = aw1[:N]                            # [512, 256]
    aw1r = np.concatenate([aw1u[k * 128:(k + 1) * 128] for k in range(KN)],
                          axis=1)             # [128, KN*AH]
    aw2 = f(inputs["aw2"])                    # [256, 1]
    aw2r = aw2[:, 0].reshape(KA, 128).T       # [128, KA]

    qw3 = f(inputs["qw3"])
    qw3p = np.zeros((H, ACTP), np.float16)
    qw3p[:, :ACT] = qw3.astype(np.float16)
    qw3r = np.ascontiguousarray(
        qw3p.reshape(KH, 128, NT, 512).transpose(2, 1, 0, 3)
            .reshape(NT, 128, KH * 512))
    qb3p = np.zeros((ACTP,), np.float32)
    qb3p[:ACT] = f(inputs["qb3"])

    shared = {
        "aw1r": h(aw1r),
        "ab1e": ab1e,
        "aw2r": h(aw2r),
        "ab2": f(inputs["ab2"]),
        "ew1": h(np.ascontiguousarray(ew1[:N])),
        "eb1e": eb1e,
        "ew2": h(f(inputs["ew2"])),
        "eb2": f(inputs["eb2"]),
        "qw1": h(f(inputs["qw1"])),
        "qb1": f(inputs["qb1"]),
        "qw2": h(f(inputs["qw2"])),
        "qb2": f(inputs["qb2"]),
        "qw3r": qw3r,
        "qb3p": qb3p,
        "ones": np.ones((1, 128), dtype=np.float32),
    }
    in_maps = []
    for c in range(NCORES):
        m = dict(shared)
        xc = x[:, c * BL:(c + 1) * BL, :]     # [L, BL, N]
        m["xT"] = np.ascontiguousarray(
            xc.transpose(2, 0, 1).reshape(N, R).astype(np.float16))
        in_maps.append(m)
    return in_maps


def run(inputs, **kwargs):
    import os
    os.environ.setdefault("BASS_NEVER_TRACE", "1")
    nc = _get_nc()
    in_maps = _prep_in_maps(inputs)
    res = run_bass_kernel_spmd(nc, in_maps, list(range(NCORES)), **kwargs)
    out = np.empty((L, B, ACT), dtype=np.float32)
    for c in range(NCORES):
        out[:, c * BL:(c + 1) * BL, :] = res.results[c]["out"]
    return out, res


def kernel(**inputs) -> np.ndarray:
    out, _ = run(inputs)
    return out


# ---------------------------------------------------------------------------
# Benchmarking: the axon tunnel costs ~11ms per dispatch, hiding sub-ms
# kernels entirely.  We amortize by repeating the kernel body inside the NEFF
# via a hardware loop (REPS>1 -> tc.For_i), then difference the pipelined
# wall times of two large rep counts, both above the dispatch floor.
# ---------------------------------------------------------------------------

_CHAIN_CACHE = {}


def _build_runner(nc_obj, in_maps):
    import jax
    from jax.sharding import Mesh, NamedSharding, PartitionSpec
    from jax.experimental.shard_map import shard_map

    import concourse.mybir as mybir_
    from concourse import bass2jax

    bass2jax.install_neuronx_cc_hook()

    partition_name = (nc_obj.partition_id_tensor.name
                      if nc_obj.partition_id_tensor else None)
    in_names, out_names, out_avals, zero_shapes = [], [], [], []
    for alloc in nc_obj.m.functions[0].allocations:
        if not isinstance(alloc, mybir_.MemoryLocationSet):
            continue
        name = alloc.memorylocations[0].name
        if alloc.kind == "ExternalInput":
            if name != partition_name:
                in_names.append(name)
        elif alloc.kind == "ExternalOutput":
            out_names.append(name)
            shape = tuple(alloc.tensor_shape)
            dtype = mybir_.dt.np(alloc.dtype)
            out_avals.append(jax.core.ShapedArray(shape, dtype))
            zero_shapes.append((shape, dtype))
    n_params = len(in_names)
    n_outs = len(out_avals)
    all_names = list(in_names) + list(out_names)
    if partition_name is not None:
        all_names.append(partition_name)

    def _body(*args):
        operands = list(args)
        if partition_name is not None:
            operands.append(bass2jax.partition_id_tensor())
        return tuple(bass2jax._bass_exec_p.bind(
            *operands,
            out_avals=tuple(out_avals),
            in_names=tuple(all_names),
            out_names=tuple(out_names),
            lowering_input_output_aliases=(),
            sim_require_finite=True,
            sim_require_nnan=True,
            nc=nc_obj,
        ))

    devices = jax.devices()[:NCORES]
    mesh = Mesh(np.asarray(devices), ("core",))
    spec = NamedSharding(mesh, PartitionSpec("core"))
    in_specs = (PartitionSpec("core"),) * (n_params + n_outs)
    out_specs = (PartitionSpec("core"),) * n_outs
    donate = tuple(range(n_params, n_params + n_outs))
    sharded = jax.jit(
        shard_map(_body, mesh=mesh, in_specs=in_specs, out_specs=out_specs,
                  check_rep=False),
        donate_argnums=donate, keep_unused=True)

    concat_in = [
        jax.device_put(
            np.concatenate([np.asarray(in_maps[c][n]) for c in range(NCORES)],
                           axis=0), spec)
        for n in in_names
    ]

    def mk_zeros():
        import jax.numpy as jnp
        return [jax.device_put(jnp.zeros((NCORES * s[0], *s[1:]), dt), spec)
                for (s, dt) in zero_shapes]

    return sharded, concat_in, mk_zeros


def bench_chain(inputs, ks=(40, 80), iters=4, rounds=5, get_nc=None,
                prep_fn=None, verbose=True):
    """Marginal per-rep NEFF time via hardware-loop rep counts.

    Both rep counts must put total exec time above the ~11ms dispatch
    floor, else the difference is corrupted by max(floor, exec).
    """
    import time
    import jax

    if get_nc is None:
        get_nc = _get_nc
    if prep_fn is None:
        prep_fn = _prep_in_maps
    entries = {}
    for k in ks:
        ck = (id(get_nc), k)
        if ck not in _CHAIN_CACHE:
            nc_obj = get_nc(k)
            in_maps = prep_fn(inputs)
            _CHAIN_CACHE[ck] = _build_runner(nc_obj, in_maps)
        entries[k] = _CHAIN_CACHE[ck]

    # warmup (compiles)
    for k in ks:
        sharded, concat_in, mk_zeros = entries[k]
        out = sharded(*concat_in, *mk_zeros())
        jax.block_until_ready(out)

    def measure(k):
        sharded, concat_in, mk_zeros = entries[k]
        zsets = [mk_zeros() for _ in range(iters)]
        jax.block_until_ready(zsets)
        t0 = time.perf_counter()
        outs = [sharded(*concat_in, *z) for z in zsets]
        jax.block_until_ready(outs)
        t1 = time.perf_counter()
        return (t1 - t0) / iters

    k1, k2 = min(ks), max(ks)
    margs = []
    for r in range(rounds):
        w1 = measure(k1)
        w2 = measure(k2)
        marg = (w2 - w1) / (k2 - k1) * 1e9
        margs.append(marg)
        if verbose:
            print(f"reps round {r}: k{k1} {w1*1e3:.2f} ms  k{k2} {w2*1e3:.2f} ms"
                  f"  marginal {marg:.0f} ns")
    margs.sort()
    return margs[len(margs) // 2]


# revision 17
# speedup vs baseline: 1.0132x; 1.0132x over previous
"""DRQN fused kernel for 8 TRN2 NeuronCores.

Data-parallel over batch B=1024 -> 128 rows per core, L=6 timesteps.
Per core the whole net runs locally (no collectives):

    inp = concat(x, pos_onehot)      -- pos is constant (l0=0), so it is
                                        folded into the first-layer biases
    att = sigmoid(relu(inp@aw1+b)@aw2+b)       [768, 1]
    enc = relu(inp@ew1+b)@ew2+b                [768, 1024]
    out = cumsum_L(enc*att)                    [768, 1024]
    act = relu(relu(out@qw1+b)@qw2+b)@qw3+b    [768, 12972]

All weights and activations are fp16 (PE runs fp16 at the same rate as
f32r, so this halves HBM/SBUF traffic at no compute cost; fp16's 10-bit
mantissa keeps the error ~1e-3 against the 2e-2 gate).  PSUM accumulation
is fp32; the gate + cumsum chain is kept in fp32 and rounded to fp16 once.
x is transposed to [feature, row] on the host so no PE transposes are
needed.  qw3 is re-tiled on the host into [26, 128, 8*512] fp16 blobs so
each column tile is one contiguous 8KB/partition DMA.

Per-core roofline: 26.2 GFLOP -> ~334 us on the PE at 2.4 GHz full rate;
~74 MB of HBM traffic -> ~207 us at 358 GB/s.  PE-bound.
"""

import numpy as np

import concourse.bass as bass
import concourse.mybir as mybir
from concourse import bacc
from concourse.bass_utils import run_bass_kernel_spmd
from concourse.tile import TileContext

F32 = mybir.dt.float32
F32R = mybir.dt.float32r
F16 = mybir.dt.float16

L, B, N = 6, 1024, 512
G = 6
H, AH, ACT = 1024, 256, 12972
NCORES = 8
BL = B // NCORES          # 128 rows of batch per core
R = L * BL                # 768 rows per core
RG = 2                    # row groups
RGS = R // RG             # 384 rows per group

KN = N // 128             # 4 K-chunks of x features
KH = H // 128             # 8 chunks of hidden features
KA = AH // 128            # 2 chunks of attention features
MROW = R // 128           # 6 row chunks (row chunk m == timestep l)

NT = 26                   # qw3 column tiles of 512 (last holds 172 valid)
ACTP = NT * 512           # 13312 padded action dim
LAST_NN = ACT - (NT - 1) * 512   # 172

BUFS = dict(mm=4, att=2, qw3s=8, qb3r=2, qb3bc=8, pq=8, st=8)
Q3_GRP = 2
REPS = 1
# truncation point for phase timing: loads|att|enc1|enc2|q1|q2|nostore|full
PHASE = "full"
_PHASES = ["loads", "att", "enc1", "enc2", "q1", "q2", "nostore", "full"]
# timing-only experiment knobs (break correctness; never set for grading)
Q3_NO_DVE = False   # skip the bias-add DVE drains in q3
Q3_NO_DMA = False   # reuse one resident wts tile instead of streaming qw3


def build_nc():
    nc = bacc.Bacc()

    xt_h = nc.declare_dram_parameter("xT", [N, R], F16, isOutput=False)
    aw1_h = nc.declare_dram_parameter("aw1r", [128, KN * AH], F16, isOutput=False)
    ab1_h = nc.declare_dram_parameter("ab1e", [AH], F32, isOutput=False)
    aw2_h = nc.declare_dram_parameter("aw2r", [128, KA], F16, isOutput=False)
    ab2_h = nc.declare_dram_parameter("ab2", [1], F32, isOutput=False)
    ew1_h = nc.declare_dram_parameter("ew1", [N, H], F16, isOutput=False)
    eb1_h = nc.declare_dram_parameter("eb1e", [H], F32, isOutput=False)
    ew2_h = nc.declare_dram_parameter("ew2", [H, H], F16, isOutput=False)
    eb2_h = nc.declare_dram_parameter("eb2", [H], F32, isOutput=False)
    qw1_h = nc.declare_dram_parameter("qw1", [H, H], F16, isOutput=False)
    qb1_h = nc.declare_dram_parameter("qb1", [H], F32, isOutput=False)
    qw2_h = nc.declare_dram_parameter("qw2", [H, H], F16, isOutput=False)
    qb2_h = nc.declare_dram_parameter("qb2", [H], F32, isOutput=False)
    qw3_h = nc.declare_dram_parameter("qw3r", [NT, 128, KH * 512], F16,
                                      isOutput=False)
    qb3_h = nc.declare_dram_parameter("qb3bc", [128, ACTP], F16, isOutput=False)
    ones_h = nc.declare_dram_parameter("ones", [1, 128], F32R, isOutput=False)
    out_h = nc.declare_dram_parameter("out", [L, BL, ACT], F16, isOutput=True)

    from contextlib import nullcontext
    with TileContext(nc) as tc:
      # REPS>1 wraps the body in a hardware loop — used only for timing
      # (amortizes the ~11ms per-dispatch tunnel overhead); the graded
      # path builds with REPS=1 and no loop.
      with (tc.For_i(0, REPS) if REPS > 1 else nullcontext()):
        persist = tc.alloc_tile_pool(name="persist", bufs=1)
        ps_a = tc.alloc_tile_pool(name="ps_a", bufs=1, space="PSUM")
        # released in reverse alloc order (LIFO per memory space)
        pool_h = tc.alloc_tile_pool(name="pool_h", bufs=1)   # h1t, qw2
        pool_g = tc.alloc_tile_pool(name="pool_g", bufs=1)   # gt, gtb, qw1
        pool_e = tc.alloc_tile_pool(name="pool_e", bufs=1)   # e1t, ew2
        pool_x = tc.alloc_tile_pool(name="pool_x", bufs=1)   # xt, aw1, ew1, a1t

        # ---- constants / biases ----
        ones_t = persist.tile([1, 128], F32R, name="ones_t")
        nc.sync.dma_start(out=ones_t, in_=ones_h[:])
        ab1_t = persist.tile([128, KA], F32, name="ab1_t")
        nc.sync.dma_start(out=ab1_t, in_=ab1_h[:].rearrange("(m p) -> p m", p=128))
        eb1_t = persist.tile([128, KH], F32, name="eb1_t")
        nc.sync.dma_start(out=eb1_t, in_=eb1_h[:].rearrange("(m p) -> p m", p=128))
        eb2_t = persist.tile([128, KH], F32, name="eb2_t")
        nc.sync.dma_start(out=eb2_t, in_=eb2_h[:].rearrange("(m p) -> p m", p=128))
        qb1_t = persist.tile([128, KH], F32, name="qb1_t")
        nc.sync.dma_start(out=qb1_t, in_=qb1_h[:].rearrange("(m p) -> p m", p=128))
        qb2_t = persist.tile([128, KH], F32, name="qb2_t")
        nc.sync.dma_start(out=qb2_t, in_=qb2_h[:].rearrange("(m p) -> p m", p=128))
        ab2_t = persist.tile([1, 1], F32, name="ab2_t")
        nc.sync.dma_start(out=ab2_t, in_=ab2_h[:])

        ph = _PHASES.index(PHASE)

        att_s = persist.tile([1, R], F32R, name="att_s")
        att_bc = persist.tile([128, R], F32, name="att_bc")
        h2t = [persist.tile([128, R], F16, name=f"h2t{k}", tag="h2t", bufs=KH)
               for k in range(KH)]

        # ---- input / weight loads (x is pre-transposed on the host) ----
        xt = [pool_x.tile([128, R], F16, name=f"xt{k}", tag="xt", bufs=KN)
              for k in range(KN)]
        for k in range(KN):
            nc.sync.dma_start(out=xt[k], in_=xt_h[k * 128:(k + 1) * 128, :])

        aw1_t = pool_x.tile([128, KN * AH], F16, name="aw1_t")
        nc.sync.dma_start(out=aw1_t, in_=aw1_h[:])
        aw2_t = pool_x.tile([128, KA], F16, name="aw2_t")
        nc.sync.dma_start(out=aw2_t, in_=aw2_h[:])
        ew1_t = [pool_x.tile([128, H], F16, name=f"ew1t{k}", tag="ew1t", bufs=KN)
                 for k in range(KN)]
        for k in range(KN):
            nc.sync.dma_start(out=ew1_t[k], in_=ew1_h[k * 128:(k + 1) * 128, :])
        ew2_t = [pool_e.tile([128, H], F16, name=f"ew2t{k}", tag="ew2t", bufs=KH)
                 for k in range(KH)]
        for k in range(KH):
            nc.sync.dma_start(out=ew2_t[k], in_=ew2_h[k * 128:(k + 1) * 128, :])
        qw1_t = [pool_g.tile([128, H], F16, name=f"qw1t{k}", tag="qw1t", bufs=KH)
                 for k in range(KH)]
        for k in range(KH):
            nc.sync.dma_start(out=qw1_t[k], in_=qw1_h[k * 128:(k + 1) * 128, :])
        qw2_t = [pool_h.tile([128, H], F16, name=f"qw2t{k}", tag="qw2t", bufs=KH)
                 for k in range(KH)]
        for k in range(KH):
            nc.sync.dma_start(out=qw2_t[k], in_=qw2_h[k * 128:(k + 1) * 128, :])

        # ---- attention branch layer 1 ----
        a1t = [pool_x.tile([128, R], F16, name=f"a1t{m}", tag="a1t", bufs=KA)
               for m in range(KA)]
        for m in (range(KA) if ph >= 1 else []):
            pls = [ps_a.tile([128, RGS], F32, name="a1ps", tag="mm", bufs=BUFS["mm"])
                   for _ in range(RG)]
            for k in range(KN):
                for g in range(RG):
                    nc.tensor.matmul(
                        pls[g], aw1_t[:, k * AH + m * 128:k * AH + (m + 1) * 128],
                        xt[k][:, g * RGS:(g + 1) * RGS],
                        start=(k == 0), stop=(k == KN - 1))
            for g in range(RG):
                nc.scalar.activation(
                    a1t[m][:, g * RGS:(g + 1) * RGS], pls[g],
                    mybir.ActivationFunctionType.Relu, bias=ab1_t[:, m:m + 1])

        # ---- attention layer 2 + sigmoid + broadcast ----
        for g in (range(RG) if ph >= 1 else []):
            aps = ps_a.tile([1, RGS], F32, name="aps", tag="att", bufs=BUFS["att"])
            for k in range(KA):
                nc.tensor.matmul(
                    aps, aw2_t[:, k:k + 1], a1t[k][:, g * RGS:(g + 1) * RGS],
                    start=(k == 0), stop=(k == KA - 1))
            nc.scalar.activation(
                att_s[:, g * RGS:(g + 1) * RGS], aps,
                mybir.ActivationFunctionType.Sigmoid, bias=ab2_t[:, 0:1])
            bps = ps_a.tile([128, RGS], F32, name="bps", tag="att", bufs=BUFS["att"])
            nc.tensor.matmul(
                bps, ones_t, att_s[:, g * RGS:(g + 1) * RGS],
                start=True, stop=True)
            nc.vector.tensor_copy(att_bc[:, g * RGS:(g + 1) * RGS], bps)

        # ---- encoder layer 1 ----
        e1t = [pool_e.tile([128, R], F16, name=f"e1t{m}", tag="e1t", bufs=KH)
               for m in range(KH)]
        for m in (range(KH) if ph >= 2 else []):
            pls = [ps_a.tile([128, RGS], F32, name="e1ps", tag="mm", bufs=BUFS["mm"])
                   for _ in range(RG)]
            for k in range(KN):
                for g in range(RG):
                    nc.tensor.matmul(
                        pls[g], ew1_t[k][:, m * 128:(m + 1) * 128],
                        xt[k][:, g * RGS:(g + 1) * RGS],
                        start=(k == 0), stop=(k == KN - 1))
            for g in range(RG):
                nc.scalar.activation(
                    e1t[m][:, g * RGS:(g + 1) * RGS], pls[g],
                    mybir.ActivationFunctionType.Relu, bias=eb1_t[:, m:m + 1])

        pool_x.release()

        # ---- encoder layer 2 + gate + cumsum over L (fp32 master) ----
        gt = [pool_g.tile([128, R], F32, name=f"gt{m}", tag="gt", bufs=KH)
              for m in range(KH)]
        gtb = [pool_g.tile([128, R], F16, name=f"gtb{m}", tag="gtb", bufs=KH)
               for m in range(KH)]
        for m in (range(KH) if ph >= 3 else []):
            pls = [ps_a.tile([128, RGS], F32, name="e2ps", tag="mm", bufs=BUFS["mm"])
                   for _ in range(RG)]
            for k in range(KH):
                for g in range(RG):
                    nc.tensor.matmul(
                        pls[g], ew2_t[k][:, m * 128:(m + 1) * 128],
                        e1t[k][:, g * RGS:(g + 1) * RGS],
                        start=(k == 0), stop=(k == KH - 1))
            for g in range(RG):
                # gt = (psum + eb2) * att
                nc.vector.scalar_tensor_tensor(
                    gt[m][:, g * RGS:(g + 1) * RGS], pls[g], eb2_t[:, m:m + 1],
                    att_bc[:, g * RGS:(g + 1) * RGS],
                    op0=mybir.AluOpType.add, op1=mybir.AluOpType.mult)
            # running sum over the 6 timesteps (128-row blocks of free dim)
            for l in range(1, L):
                nc.vector.tensor_add(
                    gt[m][:, l * 128:(l + 1) * 128],
                    gt[m][:, l * 128:(l + 1) * 128],
                    gt[m][:, (l - 1) * 128:l * 128])
            nc.scalar.copy(gtb[m], gt[m])

        pool_e.release()

        # ---- q head layer 1 ----
        h1t = [pool_h.tile([128, R], F16, name=f"h1t{m}", tag="h1t", bufs=KH)
               for m in range(KH)]
        for m in (range(KH) if ph >= 4 else []):
            pls = [ps_a.tile([128, RGS], F32, name="q1ps", tag="mm", bufs=BUFS["mm"])
                   for _ in range(RG)]
            for k in range(KH):
                for g in range(RG):
                    nc.tensor.matmul(
                        pls[g], qw1_t[k][:, m * 128:(m + 1) * 128],
                        gtb[k][:, g * RGS:(g + 1) * RGS],
                        start=(k == 0), stop=(k == KH - 1))
            for g in range(RG):
                nc.scalar.activation(
                    h1t[m][:, g * RGS:(g + 1) * RGS], pls[g],
                    mybir.ActivationFunctionType.Relu, bias=qb1_t[:, m:m + 1])

        pool_g.release()

        # ---- q head layer 2 ----
        for m in (range(KH) if ph >= 5 else []):
            pls = [ps_a.tile([128, RGS], F32, name="q2ps", tag="mm", bufs=BUFS["mm"])
                   for _ in range(RG)]
            for k in range(KH):
                for g in range(RG):
                    nc.tensor.matmul(
                        pls[g], qw2_t[k][:, m * 128:(m + 1) * 128],
                        h1t[k][:, g * RGS:(g + 1) * RGS],
                        start=(k == 0), stop=(k == KH - 1))
            for g in range(RG):
                nc.scalar.activation(
                    h2t[m][:, g * RGS:(g + 1) * RGS], pls[g],
                    mybir.ActivationFunctionType.Relu, bias=qb2_t[:, m:m + 1])

        pool_h.release()
        ps_a.release()

        # ---- action layer: out[m*128+b, :] = h2 @ qw3 + qb3 ----
        pool_q3 = tc.alloc_tile_pool(name="pool_q3", bufs=1)
        ps_b = tc.alloc_tile_pool(name="ps_b", bufs=1, space="PSUM")

        # Column tiles processed in groups of up to 6 so consecutive MMs
        # alternate PSUM banks (same-bank accumulation chains serialize on
        # the PE) and each stationary h2t chunk is reused across the group.
        wts0 = None
        groups = []
        nt0 = 0
        while nt0 < NT:
            gw = min(Q3_GRP, NT - nt0)
            groups.append(list(range(nt0, nt0 + gw)))
            nt0 += gw
        for nts in (groups if ph >= 6 else []):
            wtss, qbcs = [], []
            for nt in nts:
                if Q3_NO_DMA:
                    if wts0 is None:
                        wts0 = pool_q3.tile([128, KH * 512], F16, name="qw3s0")
                        nc.sync.dma_start(out=wts0, in_=qw3_h[0])
                    wts = wts0
                else:
                    wts = pool_q3.tile([128, KH * 512], F16, name="qw3s",
                                       tag="qw3s", bufs=BUFS["qw3s"])
                    nc.sync.dma_start(out=wts, in_=qw3_h[nt])
                wtss.append(wts)

                qb3bc = pool_q3.tile([128, 512], F16, name="qb3bc",
                                     tag="qb3bc", bufs=BUFS["qb3bc"])
                nc.sync.dma_start(out=qb3bc,
                                  in_=qb3_h[:, nt * 512:(nt + 1) * 512])
                qbcs.append(qb3bc)

            nns = [512 if nt < NT - 1 else LAST_NN for nt in nts]
            for m in range(MROW):
                pqs = [ps_b.tile([128, 512], F32, name="pq", tag="pq",
                                 bufs=BUFS["pq"]) for _ in nts]
                for k in range(KH):
                    for j in range(len(nts)):
                        nc.tensor.matmul(
                            pqs[j][:, :nns[j]],
                            h2t[k][:, m * 128:(m + 1) * 128],
                            wtss[j][:, k * 512:k * 512 + nns[j]],
                            start=(k == 0), stop=(k == KH - 1))
                for j in (range(len(nts)) if not Q3_NO_DVE else []):
                    nt = nts[j]
                    nn = nns[j]
                    n0 = nt * 512
                    st = pool_q3.tile([128, 512], F16, name="st", tag="st",
                                      bufs=BUFS["st"])
                    nc.vector.tensor_add(st[:, :nn], pqs[j][:, :nn],
                                         qbcs[j][:, :nn])
                    if ph >= 7:
                        nc.sync.dma_start(out=out_h[m, :, n0:n0 + nn],
                                          in_=st[:, :nn])

        pool_q3.release()
        ps_b.release()
        persist.release()

    nc.finalize()
    return nc


_NC_CACHE = {}


def _get_nc(reps=1, phase="full"):
    global REPS, PHASE
    key = (reps, phase)
    if key not in _NC_CACHE:
        old, oldp = REPS, PHASE
        REPS, PHASE = reps, phase
        try:
            _NC_CACHE[key] = build_nc()
        finally:
            REPS, PHASE = old, oldp
    return _NC_CACHE[key]


def _prep_in_maps(inputs):
    f = lambda a: np.ascontiguousarray(np.asarray(a, dtype=np.float32))
    h = lambda a: np.ascontiguousarray(a.astype(np.float16))
    x = f(inputs["x"])
    aw1 = f(inputs["aw1"])
    ab1e = f(inputs["ab1"]) + aw1[N]          # fold pos one-hot (l0=0)
    ew1 = f(inputs["ew1"])
    eb1e = f(inputs["eb1"]) + ew1[N]

    aw1u = aw1[:N]                            # [512, 256]
    aw1r = np.concatenate([aw1u[k * 128:(k + 1) * 128] for k in range(KN)],
                          axis=1)             # [128, KN*AH]
    aw2 = f(inputs["aw2"])                    # [256, 1]
    aw2r = aw2[:, 0].reshape(KA, 128).T       # [128, KA]

    qw3 = f(inputs["qw3"])
    qw3p = np.zeros((H, ACTP), np.float16)
    qw3p[:, :ACT] = qw3.astype(np.float16)
    qw3r = np.ascontiguousarray(
        qw3p.reshape(KH, 128, NT, 512).transpose(2, 1, 0, 3)
            .reshape(NT, 128, KH * 512))
    qb3p = np.zeros((ACTP,), np.float16)
    qb3p[:ACT] = f(inputs["qb3"]).astype(np.float16)
    qb3bc = np.ascontiguousarray(np.broadcast_to(qb3p, (128, ACTP)))

    shared = {
        "aw1r": h(aw1r),
        "ab1e": ab1e,
        "aw2r": h(aw2r),
        "ab2": f(inputs["ab2"]),
        "ew1": h(np.ascontiguousarray(ew1[:N])),
        "eb1e": eb1e,
        "ew2": h(f(inputs["ew2"])),
        "eb2": f(inputs["eb2"]),
        "qw1": h(f(inputs["qw1"])),
        "qb1": f(inputs["qb1"]),
        "qw2": h(f(inputs["qw2"])),
        "qb2": f(inputs["qb2"]),
        "qw3r": qw3r,
        "qb3bc": qb3bc,
        "ones": np.ones((1, 128), dtype=np.float32),
    }
    in_maps = []
    for c in range(NCORES):
        m = dict(shared)
        xc = x[:, c * BL:(c + 1) * BL, :]     # [L, BL, N]
        m["xT"] = np.ascontiguousarray(
            xc.transpose(2, 0, 1).reshape(N, R).astype(np.float16))
        in_maps.append(m)
    return in_maps


def run(inputs, **kwargs):
    import os
    os.environ.setdefault("BASS_NEVER_TRACE", "1")
    nc = _get_nc()
    in_maps = _prep_in_maps(inputs)
    res = run_bass_kernel_spmd(nc, in_maps, list(range(NCORES)), **kwargs)
    out = np.empty((L, B, ACT), dtype=np.float32)
    for c in range(NCORES):
        out[:, c * BL:(c + 1) * BL, :] = res.results[c]["out"].astype(np.float32)
    return out, res


def kernel(**inputs) -> np.ndarray:
    out, _ = run(inputs)
    return out


# ---------------------------------------------------------------------------
# Benchmarking: the axon tunnel costs ~11ms per dispatch, hiding sub-ms
# kernels entirely.  We amortize by repeating the kernel body inside the NEFF
# via a hardware loop (REPS>1 -> tc.For_i), then difference the pipelined
# wall times of two large rep counts, both above the dispatch floor.
# ---------------------------------------------------------------------------

_CHAIN_CACHE = {}


def _build_runner(nc_obj, in_maps):
    import jax
    from jax.sharding import Mesh, NamedSharding, PartitionSpec
    from jax.experimental.shard_map import shard_map

    import concourse.mybir as mybir_
    from concourse import bass2jax

    bass2jax.install_neuronx_cc_hook()

    partition_name = (nc_obj.partition_id_tensor.name
                      if nc_obj.partition_id_tensor else None)
    in_names, out_names, out_avals, zero_shapes = [], [], [], []
    for alloc in nc_obj.m.functions[0].allocations:
        if not isinstance(alloc, mybir_.MemoryLocationSet):
            continue
        name = alloc.memorylocations[0].name
        if alloc.kind == "ExternalInput":
            if name != partition_name:
                in_names.append(name)
        elif alloc.kind == "ExternalOutput":
            out_names.append(name)
            shape = tuple(alloc.tensor_shape)
            dtype = mybir_.dt.np(alloc.dtype)
            out_avals.append(jax.core.ShapedArray(shape, dtype))
            zero_shapes.append((shape, dtype))
    n_params = len(in_names)
    n_outs = len(out_avals)
    all_names = list(in_names) + list(out_names)
    if partition_name is not None:
        all_names.append(partition_name)

    def _body(*args):
        operands = list(args)
        if partition_name is not None:
            operands.append(bass2jax.partition_id_tensor())
        return tuple(bass2jax._bass_exec_p.bind(
            *operands,
            out_avals=tuple(out_avals),
            in_names=tuple(all_names),
            out_names=tuple(out_names),
            lowering_input_output_aliases=(),
            sim_require_finite=True,
            sim_require_nnan=True,
            nc=nc_obj,
        ))

    devices = jax.devices()[:NCORES]
    mesh = Mesh(np.asarray(devices), ("core",))
    spec = NamedSharding(mesh, PartitionSpec("core"))
    in_specs = (PartitionSpec("core"),) * (n_params + n_outs)
    out_specs = (PartitionSpec("core"),) * n_outs
    donate = tuple(range(n_params, n_params + n_outs))
    sharded = jax.jit(
        shard_map(_body, mesh=mesh, in_specs=in_specs, out_specs=out_specs,
                  check_rep=False),
        donate_argnums=donate, keep_unused=True)

    concat_in = [
        jax.device_put(
            np.concatenate([np.asarray(in_maps[c][n]) for c in range(NCORES)],
                           axis=0), spec)
        for n in in_names
    ]

    def mk_zeros():
        import jax.numpy as jnp
        return [jax.device_put(jnp.zeros((NCORES * s[0], *s[1:]), dt), spec)
                for (s, dt) in zero_shapes]

    return sharded, concat_in, mk_zeros


def bench_chain(inputs, ks=(40, 80), iters=4, rounds=5, get_nc=None,
                prep_fn=None, verbose=True):
    """Marginal per-rep NEFF time via hardware-loop rep counts.

    Both rep counts must put total exec time above the ~11ms dispatch
    floor, else the difference is corrupted by max(floor, exec).
    """
    import time
    import jax

    if get_nc is None:
        get_nc = _get_nc
    if prep_fn is None:
        prep_fn = _prep_in_maps
    entries = {}
    for k in ks:
        ck = (id(get_nc), k)
        if ck not in _CHAIN_CACHE:
            nc_obj = get_nc(k)
            in_maps = prep_fn(inputs)
            _CHAIN_CACHE[ck] = _build_runner(nc_obj, in_maps)
        entries[k] = _CHAIN_CACHE[ck]

    # warmup (compiles)
    for k in ks:
        sharded, concat_in, mk_zeros = entries[k]
        out = sharded(*concat_in, *mk_zeros())
        jax.block_until_ready(out)

    def measure(k):
        sharded, concat_in, mk_zeros = entries[k]
        zsets = [mk_zeros() for _ in range(iters)]
        jax.block_until_ready(zsets)
        t0 = time.perf_counter()
        outs = [sharded(*concat_in, *z) for z in zsets]
        jax.block_until_ready(outs)
        t1 = time.perf_counter()
        return (t1 - t0) / iters

    k1, k2 = min(ks), max(ks)
    margs = []
    for r in range(rounds):
        w1 = measure(k1)
        w2 = measure(k2)
        marg = (w2 - w1) / (k2 - k1) * 1e9
        margs.append(marg)
        if verbose:
            print(f"reps round {r}: k{k1} {w1*1e3:.2f} ms  k{k2} {w2*1e3:.2f} ms"
                  f"  marginal {marg:.0f} ns")
    margs.sort()
    return margs[len(margs) // 2]
